# revision 17
# baseline (speedup 1.0000x reference)
"""Multi-head attention kernel for Trainium2, 8 NeuronCores.

Problem (NHEAD=8, T=S=1024, B=8, A=512, hd=64):
  q = queries.reshape(T, B*NH, hd); k = keys.reshape(S, B*NH, hd)
  w = softmax(mask(q @ k^T / sqrt(hd)))      per n = b*NH + h, mask = attn_mask[n % NH]
  out = (w @ k).reshape(T, B, A)             (keys double as values)

Sharding: head-parallel. Core c owns head h=c for all 8 batches; every
problem on core c uses the single mask slice attn_mask[c] (n % 8 == h).

Per-core dataflow (bf16 matmuls, f32 PSUM; PE pinned at 1.2 GHz):
  One problem (batch) b at a time, 4 rounds of two s-tiles each. The two
  mm1 matmuls of a round target disjoint PE row groups (tile_position
  (0,0) / (64,0), K=64 each, q rows duplicated into partitions 64-127 on
  host) so they stream CONCURRENTLY - mm1 takes ~1024 not ~2048 col
  cycles per round. Three rotating 2-bank PSUM score buffers decouple
  mm1 from the exp that drains them (2 buffers provably re-serialize
  the pair through ACT). mm2 for round n is emitted after mm1 of round
  n+1 so the PE never waits on the exp->mask latency.
  mm2 accumulates [t, hd|denom] per problem in a 2-bank accumulator:
  tt 0-6 at col tt*65, tt 7 at col 512 (a 65-wide block may not cross
  the 2KB PSUM bank boundary). Output normalized to bf16 and DMA'd out
  per problem, so writeback overlaps compute for the whole run.
"""

import os
import numpy as np
import ml_dtypes

import concourse.bass as bass
import concourse.mybir as mybir
import concourse.tile as tile
from concourse.bass_utils import run_bass_kernel_spmd
from concourse.instruction_name_ordered_set import InstructionNameOrderedSet

BF16 = ml_dtypes.bfloat16

T = 1024
S = 1024
B = 8
NH = 8
HD = 64
N_CORES = 8
SCALE = 1.0 / 8.0  # 1/sqrt(hd)
MM1_N = int(os.environ.get("MM1_N", "512"))  # mm1 moving width (512; 1024 fails walrus ISA check)


# Empirical per-instruction sem-wait limit for this walrus build: even a
# Matmult with 2 waits fails codegen ("Too many sync wait commands"), so
# every instruction keeps at most one inline wait.
def _split_excess_waits(nc, default_max=1):
    """Hoist excess sem waits off instructions onto standalone
    EventSemaphore waits placed just before them on the same engine queue -
    semantically identical, since each engine executes its queue in order."""
    n = 0
    for f in nc.m.functions:
        for bb in f.blocks:
            insts = bb.instructions
            out = []
            changed = False
            for ins in insts:
                si = ins.sync_info
                waits = list(si.on_wait) if si is not None and si.on_wait else []
                if len(waits) > default_max and type(ins).__name__ != "InstEventSemaphore":
                    changed = True
                    for w in waits[:-default_max]:
                        n += 1
                        we = mybir.InstEventSemaphore(
                            name=f"WSPLIT-{n}", ins=[], outs=[]
                        )
                        we.engine = ins.engine
                        we.sync_info = mybir.SyncInfo(on_wait=[w], on_update=[])
                        nc.register_instruction(we)
                        out.append(we)
                    ins.sync_info = mybir.SyncInfo(
                        on_wait=waits[-default_max:],
                        on_update=list(si.on_update) if si.on_update else [],
                    )
                out.append(ins)
            if changed:
                bb.instructions = out


def build_nc():
    fp32 = mybir.dt.float32
    bf16 = mybir.dt.bfloat16

    nc = bass.Bass(target_bir_lowering=False)
    # Host pre-slices per core and duplicates q/k head rows into both PE
    # row-group halves: qkt row-block b holds q_b^T in partitions 0-63 AND
    # 64-127 (so tile_position (64,0) matmuls read partitions 64-127).
    qt_in = nc.dram_tensor("qt", [B * 128, T], bf16, kind="ExternalInput")
    kt_in = nc.dram_tensor("kt", [B * 128, S], bf16, kind="ExternalInput")
    knat = nc.dram_tensor("knat", [S, B * HD], bf16, kind="ExternalInput")
    maskt = nc.dram_tensor("maskt", [S, T], bf16, kind="ExternalInput")
    out = nc.dram_tensor("out", [T, B * HD], bf16, kind="ExternalOutput")

    knat3 = knat.rearrange("(st p) (b h) -> st p b h", p=128, b=B)
    out3 = out.rearrange("(tt p) (b h) -> b p tt h", p=128, b=B)

    with tile.TileContext(nc) as tc:
        with (
            tc.tile_pool(name="consts", bufs=1) as consts,
            tc.tile_pool(name="ptp", bufs=6) as ptp,
            tc.tile_pool(name="pte", bufs=6) as pte,
            tc.tile_pool(name="rcp", bufs=2) as rcp,
            tc.tile_pool(name="otp", bufs=2) as otp,
            tc.tile_pool(name="scp", bufs=3, space="PSUM") as scp,
            tc.tile_pool(name="opp", bufs=1, space="PSUM") as opp,
        ):
            # warm the ACT exp table during the DMA preamble
            wsrc = consts.tile([128, 1], fp32, tag="wsrc", name="wsrc")
            wdst = consts.tile([128, 1], bf16, tag="wdst", name="wdst")
            nc.vector.memset(wsrc[:], 0.0)
            nc.scalar.activation(wdst[:], wsrc[:], mybir.ActivationFunctionType.Exp)

            # --- resident tiles, DMA'd in consumption order ----------------
            qt = [consts.tile([128, T], bf16, tag=f"qt{b}", name=f"qt{b}") for b in range(B)]
            kt = [consts.tile([128, S], bf16, tag=f"kt{b}", name=f"kt{b}") for b in range(B)]
            mt = [consts.tile([128, T], bf16, tag=f"mt{s}", name=f"mt{s}") for s in range(8)]
            kn = [
                consts.tile([128, B, HD + 1], bf16, tag=f"kn{s}", name=f"kn{s}")
                for s in range(8)
            ]

            # problem 0's q/k first so mm1 starts immediately, then mask and
            # k-nat tiles in s-tile order, then the remaining problems' q/k.
            nc.sync.dma_start(out=qt[0][:], in_=qt_in[0:128, :])
            nc.sync.dma_start(out=kt[0][:], in_=kt_in[0:128, :])
            for st in range(8):
                nc.sync.dma_start(out=mt[st][:], in_=maskt[st * 128 : (st + 1) * 128, :])
                nc.vector.memset(kn[st][:, :, HD], 1.0)
                nc.sync.dma_start(out=kn[st][:, :, 0:HD], in_=knat3[st])
            for b in range(1, B):
                nc.sync.dma_start(out=qt[b][:], in_=qt_in[b * 128 : (b + 1) * 128, :])
                nc.sync.dma_start(out=kt[b][:], in_=kt_in[b * 128 : (b + 1) * 128, :])

            # mm2 accumulator layout inside a [128, 1024] (2-bank) tile:
            # tt 0-6 -> 65-wide blocks at tt*65 (<=455+65=520... block 6 ends
            # at 454; all within bank 0 plus into bank 1? no: 6*65+65=455).
            # Columns: tt*65 for tt in 0..6 occupy 0..454 (bank 0 is 512 f32),
            # tt 7 at 512 starts bank 1. Nothing crosses a bank boundary.
            OFF = [tt * 65 for tt in range(7)] + [512]

            def emit_mm1(b, r):
                """Two concurrent mm1 matmuls for s-tiles 2r, 2r+1 into
                rotating sc tiles (distinct PE row groups)."""
                # Interleave the two row-group streams chunk by chunk
                # (A0,B0,A1,B1): matmuls execute in order, and only adjacent
                # matmuls with disjoint row groups run concurrently.
                scs = [
                    (
                        2 * r + half,
                        scp.tile(
                            [128, 1024], fp32, tag="sc", name=f"sc_{b}_{2*r+half}"
                        ),
                    )
                    for half in range(2)
                ]
                mm1_names = InstructionNameOrderedSet()
                for i in range(0, 1024, MM1_N):
                    for half, (st, sc) in enumerate(scs):
                        lo = half * 64
                        inst = nc.tensor.matmul(
                            sc[:, i : i + MM1_N],
                            kt[b][lo : lo + 64, st * 128 : (st + 1) * 128],
                            qt[b][lo : lo + 64, i : i + MM1_N],
                            start=True,
                            stop=True,
                            tile_position=(lo, 0),
                        )
                        mm1_names.add(inst.ins.name)
                return scs, mm1_names

            def emit_exp_mask(b, scs):
                pts = []
                for st, sc in scs:
                    pe = pte.tile([128, 1024], bf16, tag="pe", name=f"pe_{b}_{st}")
                    nc.scalar.activation(
                        pe[:], sc[:], mybir.ActivationFunctionType.Exp, scale=SCALE
                    )
                    pt = ptp.tile([128, 1024], bf16, tag="pt", name=f"pt_{b}_{st}")
                    nc.vector.tensor_tensor(
                        out=pt[:], in0=pe[:], in1=mt[st][:], op=mybir.AluOpType.mult
                    )
                    pts.append((st, pt))
                return pts

            def emit_mm2(b, ops, pts, first, after=None):
                # `after`: ordering-only (nosync) deps on the current round's
                # mm1 matmuls, so the list scheduler cannot float this burst
                # ahead of them on the in-order PE queue. No runtime cost.
                for st, pt in pts:
                    for tt in range(8):
                        inst = nc.tensor.matmul(
                            ops[:, OFF[tt] : OFF[tt] + 65],
                            pt[:, tt * 128 : (tt + 1) * 128],
                            kn[st][:, b, :],
                            start=(first and st % 2 == 0 and tt in (0, 7)),
                            stop=(st == 7),
                            skip_group_check=True,
                        )
                        if after is not None:
                            inst.ins.add_nosync_dependencies_from(after)

            def emit_norm(b, ops):
                rc = rcp.tile([128, 8, 1], fp32, tag="rc", name=f"rc_{b}")
                ops7 = ops[:, 0 : 7 * 65].rearrange("p (tt x) -> p tt x", x=65)
                nc.vector.reciprocal(rc[:, 0:7, 0], ops7[:, :, HD])
                nc.vector.reciprocal(rc[:, 7, 0:1], ops[:, 512 + HD : 512 + HD + 1])
                ot = otp.tile([128, 8, HD], bf16, tag="ot", name=f"ot_{b}")
                nc.vector.tensor_tensor(
                    out=ot[:, 0:7, :],
                    in0=ops7[:, :, 0:HD],
                    in1=rc[:, 0:7, :].to_broadcast([128, 7, HD]),
                    op=mybir.AluOpType.mult,
                )
                nc.vector.tensor_tensor(
                    out=ot[:, 7, :],
                    in0=ops[:, 512 : 512 + HD],
                    in1=rc[:, 7, :].to_broadcast([128, HD]),
                    op=mybir.AluOpType.mult,
                )
                nc.sync.dma_start(out=out3[b], in_=ot[:])

            # --- main loop: 32 rounds, mm2 trails mm1 by one round. The
            # accumulator for problem pb is allocated when its first mm2 is
            # emitted (after problem pb-1's norm), so the single-buffer pool
            # sees a clean write-after-read ordering.
            prev = None  # (pb, pr, ppts)
            ops_cur = None
            for n in range(32):
                b, r = divmod(n, 4)
                scs, mm1_names = emit_mm1(b, r)
                pts = emit_exp_mask(b, scs)
                if prev is not None:
                    pb, pr, ppts = prev
                    if pr == 0:
                        ops_cur = opp.tile(
                            [128, 1024], fp32, tag="ops", name=f"ops_{pb}"
                        )
                    emit_mm2(pb, ops_cur, ppts, first=(pr == 0), after=mm1_names)
                    if pr == 3:
                        emit_norm(pb, ops_cur)
                prev = (b, r, pts)
            pb, pr, ppts = prev
            emit_mm2(pb, ops_cur, ppts, first=False)
            emit_norm(pb, ops_cur)

    _split_excess_waits(nc)
    return nc


_NC_CACHE = None


def _get_nc():
    global _NC_CACHE
    if _NC_CACHE is None:
        _NC_CACHE = build_nc()
    return _NC_CACHE


def kernel(queries: np.ndarray, keys: np.ndarray, attn_mask: np.ndarray) -> np.ndarray:
    assert queries.shape == (T, B, NH * HD)
    assert keys.shape == (S, B, NH * HD)
    assert attn_mask.shape == (B, T, S)

    q_bf = np.asarray(queries, np.float32).astype(BF16)  # [T, B, A]
    k_bf = np.asarray(keys, np.float32).astype(BF16)
    m_bf = np.asarray(attn_mask).astype(BF16)  # bool -> 0.0/1.0

    in_maps = []
    for c in range(N_CORES):
        qs = q_bf[:, :, c * HD : (c + 1) * HD]  # [T, B, 64]
        ks = k_bf[:, :, c * HD : (c + 1) * HD]
        # [B, 128, T]: per problem b, q_b^T duplicated into both row halves
        qt2 = np.empty((B, 128, T), BF16)
        kt2 = np.empty((B, 128, S), BF16)
        for b in range(B):
            qT = np.ascontiguousarray(qs[:, b, :].T)
            kT = np.ascontiguousarray(ks[:, b, :].T)
            qt2[b, 0:64] = qT
            qt2[b, 64:128] = qT
            kt2[b, 0:64] = kT
            kt2[b, 64:128] = kT
        in_maps.append(
            {
                "qt": qt2.reshape(B * 128, T),
                "kt": kt2.reshape(B * 128, S),
                "knat": np.ascontiguousarray(ks.reshape(S, B * HD)),
                "maskt": np.ascontiguousarray(m_bf[c].T),
            }
        )

    nc = _get_nc()
    res = run_bass_kernel_spmd(nc, in_maps, core_ids=list(range(N_CORES)))
    kernel.last_results = res

    outp = np.empty((T, B, NH * HD), np.float32)
    for c in range(N_CORES):
        outp[:, :, c * HD : (c + 1) * HD] = (
            res.results[c]["out"].astype(np.float32).reshape(T, B, HD)
        )
    return outp


# revision 21
# speedup vs baseline: 1.1669x; 1.1669x over previous
"""Multi-head attention kernel for Trainium2, 8 NeuronCores.

Problem (NHEAD=8, T=S=1024, B=8, A=512, hd=64):
  q = queries.reshape(T, B*NH, hd); k = keys.reshape(S, B*NH, hd)
  w = softmax(mask(q @ k^T / sqrt(hd)))      per n = b*NH + h, mask = attn_mask[n % NH]
  out = (w @ k).reshape(T, B, A)             (keys double as values)

Sharding: head-parallel. Core c owns head h=c for all 8 batches; every
problem on core c uses the single mask slice attn_mask[c] (n % 8 == h).

Per-core dataflow (bf16 matmuls, f32 PSUM; PE pinned at 1.2 GHz):
  One problem (batch) b at a time, 4 rounds of two s-tiles each. The two
  mm1 matmuls of a round target disjoint PE row groups (tile_position
  (0,0) / (64,0), K=64 each, q rows duplicated into partitions 64-127 on
  host) so they stream CONCURRENTLY - mm1 takes ~1024 not ~2048 col
  cycles per round. Three rotating 2-bank PSUM score buffers decouple
  mm1 from the exp that drains them (2 buffers provably re-serialize
  the pair through ACT). mm2 for round n is emitted after mm1 of round
  n+1 so the PE never waits on the exp->mask latency.
  mm2 accumulates [t, hd|denom] per problem in a 2-bank accumulator:
  tt 0-6 at col tt*65, tt 7 at col 512 (a 65-wide block may not cross
  the 2KB PSUM bank boundary). Output normalized to bf16 and DMA'd out
  per problem, so writeback overlaps compute for the whole run.
"""

import os
import numpy as np
import ml_dtypes

import concourse.bass as bass
import concourse.mybir as mybir
import concourse.tile as tile
from concourse.bass_utils import run_bass_kernel_spmd
from concourse.instruction_name_ordered_set import InstructionNameOrderedSet

BF16 = ml_dtypes.bfloat16

T = 1024
S = 1024
B = 8
NH = 8
HD = 64
N_CORES = 8
SCALE = 1.0 / 8.0  # 1/sqrt(hd)
MM1_N = int(os.environ.get("MM1_N", "512"))  # mm1 moving width (512; 1024 fails walrus ISA check)


# Empirical per-instruction sem-wait limit for this walrus build: even a
# Matmult with 2 waits fails codegen ("Too many sync wait commands"), so
# every instruction keeps at most one inline wait.
def _split_excess_waits(nc, default_max=1):
    """Hoist excess sem waits off instructions onto standalone
    EventSemaphore waits placed just before them on the same engine queue -
    semantically identical, since each engine executes its queue in order."""
    n = 0
    for f in nc.m.functions:
        for bb in f.blocks:
            insts = bb.instructions
            out = []
            changed = False
            for ins in insts:
                si = ins.sync_info
                waits = list(si.on_wait) if si is not None and si.on_wait else []
                if len(waits) > default_max and type(ins).__name__ != "InstEventSemaphore":
                    changed = True
                    for w in waits[:-default_max]:
                        n += 1
                        we = mybir.InstEventSemaphore(
                            name=f"WSPLIT-{n}", ins=[], outs=[]
                        )
                        we.engine = ins.engine
                        we.sync_info = mybir.SyncInfo(on_wait=[w], on_update=[])
                        nc.register_instruction(we)
                        out.append(we)
                    ins.sync_info = mybir.SyncInfo(
                        on_wait=waits[-default_max:],
                        on_update=list(si.on_update) if si.on_update else [],
                    )
                out.append(ins)
            if changed:
                bb.instructions = out


def build_nc():
    fp32 = mybir.dt.float32
    bf16 = mybir.dt.bfloat16

    nc = bass.Bass(target_bir_lowering=False)
    # Host pre-slices per core and duplicates q/k head rows into both PE
    # row-group halves: qkt row-block b holds q_b^T in partitions 0-63 AND
    # 64-127 (so tile_position (64,0) matmuls read partitions 64-127).
    qt_in = nc.dram_tensor("qt", [B * 128, T], bf16, kind="ExternalInput")
    kt_in = nc.dram_tensor("kt", [B * 128, S], bf16, kind="ExternalInput")
    knat = nc.dram_tensor("knat", [S, B * HD], bf16, kind="ExternalInput")
    maskt = nc.dram_tensor("maskt", [S, T], bf16, kind="ExternalInput")
    out = nc.dram_tensor("out", [T, B * HD], bf16, kind="ExternalOutput")

    knat3 = knat.rearrange("(st p) (b h) -> st p b h", p=128, b=B)
    out3 = out.rearrange("(tt p) (b h) -> b p tt h", p=128, b=B)

    with tile.TileContext(nc) as tc:
        with (
            tc.tile_pool(name="consts", bufs=1) as consts,
            tc.tile_pool(name="ptp", bufs=6) as ptp,
            tc.tile_pool(name="pte", bufs=6) as pte,
            tc.tile_pool(name="rcp", bufs=2) as rcp,
            tc.tile_pool(name="otp", bufs=2) as otp,
            tc.tile_pool(name="scp", bufs=3, space="PSUM") as scp,
            tc.tile_pool(name="opp", bufs=1, space="PSUM") as opp,
        ):
            # warm the ACT exp table during the DMA preamble
            wsrc = consts.tile([128, 1], fp32, tag="wsrc", name="wsrc")
            wdst = consts.tile([128, 1], bf16, tag="wdst", name="wdst")
            nc.vector.memset(wsrc[:], 0.0)
            nc.scalar.activation(wdst[:], wsrc[:], mybir.ActivationFunctionType.Exp)

            # --- resident tiles, DMA'd in consumption order ----------------
            qt = [consts.tile([128, T], bf16, tag=f"qt{b}", name=f"qt{b}") for b in range(B)]
            kt = [consts.tile([128, S], bf16, tag=f"kt{b}", name=f"kt{b}") for b in range(B)]
            mt = [consts.tile([128, T], bf16, tag=f"mt{s}", name=f"mt{s}") for s in range(8)]
            kn = [
                consts.tile([128, B, HD + 1], bf16, tag=f"kn{s}", name=f"kn{s}")
                for s in range(8)
            ]

            # problem 0's q/k first so mm1 starts immediately, then mask and
            # k-nat tiles in s-tile order, then the remaining problems' q/k.
            nc.sync.dma_start(out=qt[0][:], in_=qt_in[0:128, :])
            nc.sync.dma_start(out=kt[0][:], in_=kt_in[0:128, :])
            for st in range(8):
                nc.sync.dma_start(out=mt[st][:], in_=maskt[st * 128 : (st + 1) * 128, :])
                nc.vector.memset(kn[st][:, :, HD], 1.0)
                nc.sync.dma_start(out=kn[st][:, :, 0:HD], in_=knat3[st])
            for b in range(1, B):
                nc.sync.dma_start(out=qt[b][:], in_=qt_in[b * 128 : (b + 1) * 128, :])
                nc.sync.dma_start(out=kt[b][:], in_=kt_in[b * 128 : (b + 1) * 128, :])

            # mm2 accumulator layout inside a [128, 1024] (2-bank) tile:
            # tt 0-6 -> 65-wide blocks at tt*65 (<=455+65=520... block 6 ends
            # at 454; all within bank 0 plus into bank 1? no: 6*65+65=455).
            # Columns: tt*65 for tt in 0..6 occupy 0..454 (bank 0 is 512 f32),
            # tt 7 at 512 starts bank 1. Nothing crosses a bank boundary.
            OFF = [tt * 65 for tt in range(7)] + [512]

            def emit_mm1(b, r):
                """Two concurrent mm1 matmuls for s-tiles 2r, 2r+1 into
                rotating sc tiles (distinct PE row groups)."""
                # Interleave the two row-group streams chunk by chunk
                # (A0,B0,A1,B1): matmuls execute in order, and only adjacent
                # matmuls with disjoint row groups run concurrently.
                scs = [
                    (
                        2 * r + half,
                        scp.tile(
                            [128, 1024], fp32, tag="sc", name=f"sc_{b}_{2*r+half}"
                        ),
                    )
                    for half in range(2)
                ]
                mm1_names = InstructionNameOrderedSet()
                for i in range(0, 1024, MM1_N):
                    for half, (st, sc) in enumerate(scs):
                        lo = half * 64
                        inst = nc.tensor.matmul(
                            sc[:, i : i + MM1_N],
                            kt[b][lo : lo + 64, st * 128 : (st + 1) * 128],
                            qt[b][lo : lo + 64, i : i + MM1_N],
                            start=True,
                            stop=True,
                            tile_position=(lo, 0),
                        )
                        mm1_names.add(inst.ins.name)
                return scs, mm1_names

            # Schraudolph exp on DVE: bitcast_bf16(int16(y*2^7/ln2 + 127*128-7))
            # ~= e^y for y = sc*SCALE in [-6, 6]. RMS err ~1.8% sawtooth; the
            # constant-scale part cancels in softmax. Offloading a fraction of
            # exp tiles from ACT (the saturated engine) to DVE, and a fraction
            # of mask multiplies from DVE to the otherwise-idle GpSimd, gives
            # every non-PE engine slack so the PE-paced pipeline is stable.
            SCH_A = SCALE * 128.0 / float(np.log(2.0))
            SCH_B = 127.0 * 128.0 - 7.0

            def emit_exp_mask(b, scs, n):
                pts = []
                for half, (st, sc) in enumerate(scs):
                    pt = ptp.tile([128, 1024], bf16, tag="pt", name=f"pt_{b}_{st}")
                    use_sch = half == 0 and (n % 8) in (1, 4, 6)
                    pool_mask = half == 1 and (n % 4) != 3
                    if use_sch:
                        sch = pte.tile(
                            [128, 1024], mybir.dt.int16, tag="sch", name=f"sch_{b}_{st}"
                        )
                        nc.vector.tensor_scalar(
                            out=sch[:], in0=sc[:], scalar1=SCH_A, scalar2=SCH_B,
                            op0=mybir.AluOpType.mult, op1=mybir.AluOpType.add,
                        )
                        nc.vector.tensor_tensor(
                            out=pt[:], in0=sch[:].bitcast(bf16), in1=mt[st][:],
                            op=mybir.AluOpType.mult,
                        )
                    else:
                        pe = pte.tile([128, 1024], bf16, tag="pe", name=f"pe_{b}_{st}")
                        nc.scalar.activation(
                            pe[:], sc[:], mybir.ActivationFunctionType.Exp, scale=SCALE
                        )
                        eng = nc.gpsimd if pool_mask else nc.vector
                        eng.tensor_tensor(
                            out=pt[:], in0=pe[:], in1=mt[st][:], op=mybir.AluOpType.mult
                        )
                    pts.append((st, pt))
                return pts

            def emit_mm2(b, ops, pts, first):
                for st, pt in pts:
                    for tt in range(8):
                        nc.tensor.matmul(
                            ops[:, OFF[tt] : OFF[tt] + 65],
                            pt[:, tt * 128 : (tt + 1) * 128],
                            kn[st][:, b, :],
                            start=(first and st % 2 == 0 and tt in (0, 7)),
                            stop=(st == 7),
                            skip_group_check=True,
                        )

            def emit_norm(b, ops):
                rc = rcp.tile([128, 8, 1], fp32, tag="rc", name=f"rc_{b}")
                ops7 = ops[:, 0 : 7 * 65].rearrange("p (tt x) -> p tt x", x=65)
                nc.vector.reciprocal(rc[:, 0:7, 0], ops7[:, :, HD])
                nc.vector.reciprocal(rc[:, 7, 0:1], ops[:, 512 + HD : 512 + HD + 1])
                ot = otp.tile([128, 8, HD], bf16, tag="ot", name=f"ot_{b}")
                nc.vector.tensor_tensor(
                    out=ot[:, 0:7, :],
                    in0=ops7[:, :, 0:HD],
                    in1=rc[:, 0:7, :].to_broadcast([128, 7, HD]),
                    op=mybir.AluOpType.mult,
                )
                nc.vector.tensor_tensor(
                    out=ot[:, 7, :],
                    in0=ops[:, 512 : 512 + HD],
                    in1=rc[:, 7, :].to_broadcast([128, HD]),
                    op=mybir.AluOpType.mult,
                )
                nc.sync.dma_start(out=out3[b], in_=ot[:])

            # --- main loop: 32 rounds, mm2 trails mm1 by one round. The
            # accumulator for problem pb is allocated when its first mm2 is
            # emitted (after problem pb-1's norm), so the single-buffer pool
            # sees a clean write-after-read ordering.
            prev = None  # (pb, pr, ppts)
            ops_cur = None
            for n in range(32):
                b, r = divmod(n, 4)
                scs, mm1_names = emit_mm1(b, r)
                pts = emit_exp_mask(b, scs, n)
                if prev is not None:
                    pb, pr, ppts = prev
                    if pr == 0:
                        ops_cur = opp.tile(
                            [128, 1024], fp32, tag="ops", name=f"ops_{pb}"
                        )
                    emit_mm2(pb, ops_cur, ppts, first=(pr == 0))
                    if pr == 3:
                        emit_norm(pb, ops_cur)
                prev = (b, r, pts)
            pb, pr, ppts = prev
            emit_mm2(pb, ops_cur, ppts, first=False)
            emit_norm(pb, ops_cur)

    _split_excess_waits(nc)
    return nc


_NC_CACHE = None


def _get_nc():
    global _NC_CACHE
    if _NC_CACHE is None:
        _NC_CACHE = build_nc()
    return _NC_CACHE


def kernel(queries: np.ndarray, keys: np.ndarray, attn_mask: np.ndarray) -> np.ndarray:
    assert queries.shape == (T, B, NH * HD)
    assert keys.shape == (S, B, NH * HD)
    assert attn_mask.shape == (B, T, S)

    q_bf = np.asarray(queries, np.float32).astype(BF16)  # [T, B, A]
    k_bf = np.asarray(keys, np.float32).astype(BF16)
    m_bf = np.asarray(attn_mask).astype(BF16)  # bool -> 0.0/1.0

    in_maps = []
    for c in range(N_CORES):
        qs = q_bf[:, :, c * HD : (c + 1) * HD]  # [T, B, 64]
        ks = k_bf[:, :, c * HD : (c + 1) * HD]
        # [B, 128, T]: per problem b, q_b^T duplicated into both row halves
        qt2 = np.empty((B, 128, T), BF16)
        kt2 = np.empty((B, 128, S), BF16)
        for b in range(B):
            qT = np.ascontiguousarray(qs[:, b, :].T)
            kT = np.ascontiguousarray(ks[:, b, :].T)
            qt2[b, 0:64] = qT
            qt2[b, 64:128] = qT
            kt2[b, 0:64] = kT
            kt2[b, 64:128] = kT
        in_maps.append(
            {
                "qt": qt2.reshape(B * 128, T),
                "kt": kt2.reshape(B * 128, S),
                "knat": np.ascontiguousarray(ks.reshape(S, B * HD)),
                "maskt": np.ascontiguousarray(m_bf[c].T),
            }
        )

    nc = _get_nc()
    res = run_bass_kernel_spmd(nc, in_maps, core_ids=list(range(N_CORES)))
    kernel.last_results = res

    outp = np.empty((T, B, NH * HD), np.float32)
    for c in range(N_CORES):
        outp[:, :, c * HD : (c + 1) * HD] = (
            res.results[c]["out"].astype(np.float32).reshape(T, B, HD)
        )
    return outp
